# revision 48
# baseline (speedup 1.0000x reference)
"""Multi-head attention (B=2, S=4096, D=512, H=8) on 8 NeuronCores.

Sharding: data-parallel on batch x head-pair-parallel.  Core c handles
batch b = c//4 and heads (2*(c%4), 2*(c%4)+1).  Each core computes its
[4096, 128] slice of the output; the host scatters inputs / gathers
outputs.

Per-core kernel (Bass/Tile), operands in fp16 (fp32 PSUM accumulate).
The exp over all 2*4096^2 scores is the hard bottleneck.  The scalar
(ACT) engine does it at ~0.87 ns/col; this kernel additionally routes a
fraction of the scores through the *vector* engine via a custom-DVE op
(EXP_ANT_1OP: deg-2 minimax poly of exp(s/128) + 4 chained squarings,
8 ALU stages, ~1.04 ns/col), so both engines burn down the softmax in
parallel:

  - prologue (qc0/qc1) is the baseline schedule: projections pipelined
    into the attention sweep, E@V deferred through a deep fp16 ring.
  - steady state (qc2+): per q-chunk the 64 (kc,h) scorx slices walk in
    order through two decoupled chunk streams: ACT chunks (2 slices,
    [128,1024], own 2x2-bank PSUM ring) and DVE chunks (1 slice,
    [128,512], own 2x1-bank ring), ~24/64 slices on DVE.  E@V pops and
    start/stop flags are unchanged since slice order is preserved.
  - PSUM: ACT ring 4 banks + DVE ring 2 + E@V accumulators 2 = 8.

Measured on the 8 axon trn2 cores: baseline 288.7 us (ACT ~258 us busy,
87%); this version targets ~200 us by offloading ~37% of exp to DVE.
"""

import numpy as np

N_CORES = 8
S_FULL = 4096
D_MODEL = 512
HEAD = 64

# steady-state schedule knobs.  Engine per kc: adjacent A-kc pairs run
# exp on ACT with fp8 et and are consumed by one DoubleRow (256-key)
# E@V pop per head; D-kc run exp on DVE (f16 et, plain f16 pops).
# 20 A (10 fp8 pairs) + 12 D balances ACT ~20.7us, DVE ~16.4us and cuts
# PE pops from 64 to 44 per qc.
KC_PATTERN = ('A', 'A', 'D') * 10 + ('D', 'D')
POP_Q_BY_QC = {2: 5, 3: 4}  # E@V pops per kc-chunk (default 3): drain
                            # the deferred prologue backlog fast so the
                            # et ring stops throttling qc2-4 exps
POP_Q = 3
POP_KEEP = 8          # min fifo depth: pops trail exp, never block PE

_cached = {}


def _register_exp_ops():
    """Register custom-DVE exp ops (rows 17+, free on TRN2).

    EXP_ANT_1OP: exp(0.125*s) ~= p(s)^16 with p = (s*c2 + c1)*s + c0,
    the relative-minimax deg-2 fit of exp(y) on y = s/128 in [-.215,.215]
    (raw scores |0.125*s| <= 3.44 observed <= 3.2).  4+4 = 8 ALU stages,
    1 elem/cycle/lane.  Worst rel err 1.1e-2 at the range edge, 4.7e-3
    for |z|<=2.5; end-to-end (softmax-normalized, f16 weights) the
    output delta is <2e-4 -- an order below the f16 noise floor budget.

    EXP_ANT_P1/P2: accurate 2-instruction chain (deg-3 Taylor of
    exp(s*2^-11) then 8 squarings), kept as fallback.
    """
    import concourse.dve_ops as dve_ops
    from concourse.dve_spec import Spec, Src0, C0, C1, C2, One, lower, sq, _has_src1
    from concourse.dve_uop import DveOpSpec

    have = {op.name: op for op in dve_ops.OPS}

    def reg(name, spec):
        if name in have:
            return have[name]
        row = dve_ops._CUSTOM_DVE_ROW_BASE + len(dve_ops.OPS)
        dve_ops._SUB_OPCODE_FOR_NAME[name] = row
        uops = lower(spec, ver="v3")
        sha3 = DveOpSpec(
            name=name, opcode=row, uops=uops, rd1_en=_has_src1(spec)
        ).sha("v3")
        op = dve_ops.DveOp(name, spec, subdim=False, uops_sha={"v3": sha3})
        dve_ops.OPS.append(op)
        dve_ops.CUSTOM_DVE_SPECS[name] = spec
        return op

    x = (Src0 * C2 + C1) * Src0 + C0
    for _ in range(4):
        x = sq(x)
    spec_1op = Spec(
        body=x,
        reference=lambda in0, in1, s0, s1, imm2: (
            ((in0.astype(np.float32) * np.float32(imm2) + np.float32(s1))
             * in0 + np.float32(s0)).astype(np.float32) ** 16
        ).astype(np.float32),
    )

    body1 = ((Src0 * C2 + C1) * Src0 + C0) * Src0 + One
    spec_p1 = Spec(
        body=body1,
        reference=lambda in0, in1, s0, s1, imm2: (
            ((in0.astype(np.float32) * np.float32(imm2) + np.float32(s1))
             * in0 + np.float32(s0)) * in0 + np.float32(1.0)
        ).astype(np.float32),
    )
    y = Src0
    for _ in range(8):
        y = sq(y)
    spec_p2 = Spec(
        body=y,
        reference=lambda in0, in1, s0, s1, imm2: (
            in0.astype(np.float32) ** 256
        ).astype(np.float32),
    )
    return (
        reg("EXP_ANT_1OP", spec_1op),
        reg("EXP_ANT_P1", spec_p1),
        reg("EXP_ANT_P2", spec_p2),
    )


# minimax deg-2 fit of exp(y), y = s/128, |y| <= 3.44/16; s-domain coeffs
_E1_C0 = 1.0000528961258521
_E1_C1 = 0.007848291635150783
_E1_C2 = 3.0376679783362523e-05


def build_nc(S=S_FULL):
    import concourse.bass as bass
    from concourse import bacc
    import concourse.mybir as mybir
    import concourse.tile as tile
    from concourse.masks import make_identity
    f32 = mybir.dt.float32
    f16 = mybir.dt.float16
    f8 = mybir.dt.float8e4
    AF = mybir.ActivationFunctionType

    D = D_MODEL
    n_qc = S // 512     # 512-wide query chunks
    n_kc = S // 128     # 128-wide key tiles
    n_dc = D // 128     # 128-wide contraction chunks of D

    EXP_1OP, EXP_P1, EXP_P2 = _register_exp_ops()

    nc = bacc.Bacc()

    xT = nc.dram_tensor("xT", [D, S], f16, kind="ExternalInput")
    # weights arrive in SBUF-image layout with the bias as a trailing
    # column (f16 bias rounding is far below fp16 operand noise): one
    # contiguous DMA each
    wqT = nc.dram_tensor("wqT", [128, n_dc * 128 + 1], f16, kind="ExternalInput")
    wkT = nc.dram_tensor("wkT", [128, n_dc * 128 + 1], f16, kind="ExternalInput")
    wvT = nc.dram_tensor("wvT", [128, n_dc * 130 + 130], f16,
                         kind="ExternalInput")
    out = nc.dram_tensor("out", [S, 128], f32, kind="ExternalOutput")

    with tile.TileContext(nc) as tc:
        with (
            tc.tile_pool(name="consts", bufs=1) as consts,
            tc.tile_pool(name="persist", bufs=1) as persist,
        ):
            ident = consts.tile([128, 128], f16, name="ident")
            tiny = consts.tile([128, 8], f32, name="tiny")
            tiny_o = consts.tile([128, 8], f16, name="tiny_o")
            wq_sb = consts.tile([128, n_dc * 128 + 1], f16, name="wq_sb")
            wk_sb = consts.tile([128, n_dc * 128 + 1], f16, name="wk_sb")
            wv_sb = consts.tile([128, n_dc * 130 + 130], f16, name="wv_sb")
            bq_sb = consts.tile([128, 1], f32, name="bq_sb")
            bk_sb = consts.tile([128, 1], f32, name="bk_sb")
            bvb_sb = consts.tile([128, 130], f32, name="bvb_sb")
            xt = persist.tile([128, n_dc * S], f16, name="xt")

            def xs(dc, sl):
                return xt[:, dc * S + sl.start: dc * S + sl.stop]
            qt = persist.tile([128, S], f16, name="qt")
            kt = persist.tile([128, S], f16, name="kt")
            # V1[kc*130 + h*65 : +65] = [V_h | ones] per key tile kc.
            v1 = persist.tile([128, n_kc * 130], f16, name="v1")
            # fp8 shadow of V for the DoubleRow pair-pops, pair-major:
            # col = j*320 + h*160 + k*80 + d for f8 pair j, half k, dim d<65.
            # The 80-col k-slots satisfy the dual-fp8 Ldweights ISA rule
            # (free-dim steps even and 16B-aligned).  Only A-kc live here.
            n_p8 = sum(1 for e in KC_PATTERN if e == 'A') * (n_kc // len(KC_PATTERN)) // 2
            # +160 pad so the k=1 shadow write can view a full 320-col window
            v1_8 = persist.tile([128, n_p8 * 320 + 160], f8, name="v1_8")

            # ACT table preload: dummy exp at t=0 hides the ~2.7us load.
            nc.vector.memset(tiny[:], 0.0)
            nc.scalar.activation(tiny_o[:], tiny[:], AF.Exp, scale=0.125)
            make_identity(nc, ident)

            # DMAs.  Issue order IS priority: each dma_start costs
            # 0.6-1.4us of Sync issue time and the DMA engines round-robin
            # fairly over everything outstanding, so: tiny weights first,
            # then x block0 as a single 3D transfer, then the later x
            # pieces (xB is also held behind xA by a deliberate 1-column
            # WAW overlap so block0+xA are never starved).
            x_src = xT[:, :].rearrange("(dc p) s -> p dc s", dc=n_dc)
            x_dst = xt[:].rearrange("p (dc s) -> p dc s", s=S)
            nc.sync.dma_start(wk_sb[:], wkT[:, :])
            nc.sync.dma_start(wq_sb[:], wqT[:, :])
            nc.sync.dma_start(x_dst[:, :, 0:512], x_src[:, :, 0:512])
            nc.sync.dma_start(x_dst[:, :, 512:1024], x_src[:, :, 512:1024])
            nc.sync.dma_start(wv_sb[:], wvT[:, :])
            nc.sync.dma_start(x_dst[:, :, 1024:2049], x_src[:, :, 1024:2049])
            nc.sync.dma_start(x_dst[:, :, 2048:S], x_src[:, :, 2048:S])
            # biases ride in the weight images as f16; widen to f32 once
            nc.vector.tensor_copy(bk_sb[:], wk_sb[:, n_dc * 128: n_dc * 128 + 1])
            nc.vector.tensor_copy(bq_sb[:], wq_sb[:, n_dc * 128: n_dc * 128 + 1])
            nc.vector.tensor_copy(bvb_sb[:], wv_sb[:, n_dc * 130: n_dc * 130 + 130])

            with (
                tc.tile_pool(name="etp", bufs=34) as etp,
                tc.tile_pool(name="outp", bufs=2) as outp,
            ):
                pools = {}  # 'st': pool for emit_norm's PE transposes

                # ---------- projection pieces ----------
                def emit_kq(dst, w_sb, b_sb, b, pool):
                    cs = slice(b * 512, (b + 1) * 512)
                    p = pool.tile([128, 512], f32, name="pp", tag="pp")
                    for dc in range(n_dc):
                        nc.tensor.matmul(
                            p[:],
                            lhsT=w_sb[:, dc * 128:(dc + 1) * 128],
                            rhs=xs(dc, cs),
                            start=(dc == 0),
                            stop=(dc == n_dc - 1),
                        )
                    nc.vector.tensor_scalar_add(dst[:, cs], p[:], b_sb[:])

                vq_done = [0]

                def is_f8_kc(kc):
                    return KC_PATTERN[kc % len(KC_PATTERN)] == 'A'

                def f8_pair(kc):
                    """(pair index j, half k) for an f8 kc under KC_PATTERN."""
                    per = len(KC_PATTERN)
                    n_a_per = sum(1 for e in KC_PATTERN if e == 'A')
                    idx = (kc // per) * n_a_per + sum(
                        1 for e in KC_PATTERN[:kc % per] if e == 'A')
                    return idx // 2, idx % 2

                def emit_vq(pool):
                    # V projection for the next 128-token tile.
                    st_ = vq_done[0]
                    ss = slice(st_ * 128, (st_ + 1) * 128)
                    p = pool.tile([128, 512], f32, name="pp", tag="pp")
                    for dc in range(n_dc):
                        nc.tensor.matmul(
                            p[:, 0:130],
                            lhsT=xs(dc, ss),
                            rhs=wv_sb[:, dc * 130:(dc + 1) * 130],
                            start=(dc == 0),
                            stop=(dc == n_dc - 1),
                        )
                    nc.vector.tensor_add(
                        v1[:, st_ * 130:(st_ + 1) * 130], p[:, 0:130], bvb_sb[:]
                    )
                    if is_f8_kc(st_):
                        # fp8 shadow for DoubleRow pops; col layout
                        # j*320 + h*160 + k*80 + d -> h-strided view
                        j, k = f8_pair(st_)
                        base = j * 320 + k * 80
                        dst = (v1_8[:, base: base + 320]
                               .rearrange("p (h c) -> p h c", h=2, c=160)
                               [:, :, 0:65])
                        src = (p[:, 0:130]
                               .rearrange("p (h c) -> p h c", h=2, c=65))
                        bsrc = (bvb_sb[:, 0:130]
                                .rearrange("p (h c) -> p h c", h=2, c=65))
                        nc.vector.tensor_add(dst, src, bsrc)
                    vq_done[0] += 1

                # ---------- attention ----------
                ev_fifo = []        # (qc, kc, h, et_tile, col_off)
                ev_left = {}        # qc -> slices not yet popped
                po_by_qc = {}

                def emit_norm(po, qc):
                    # res[:, t*128+h*64 : +64] = head h of output rows
                    # qc*512 + t*128 + [0:128); shipped as one 3D DMA
                    res = outp.tile([128, 512], f32, name="res", tag="res")
                    last = qc == n_qc - 1
                    ots = []
                    for h in range(2):
                        ot = outp.tile([128, 512], f16, name="ot", tag="ot")
                        if last and h == 0:
                            nc.scalar.copy(ot[:], po[h][:])
                        else:
                            nc.vector.tensor_copy(ot[:], po[h][:])
                        ots.append(ot)
                    for t in range(4):
                        for h in range(2):
                            if last:
                                pt = pools['st'].tile([128, 65], f16, name="pt",
                                                      tag="st")
                                nc.tensor.transpose(
                                    pt[:],
                                    ots[h][0:65, t * 128:(t + 1) * 128],
                                    ident[0:65, 0:65],
                                )
                                src = pt
                            else:
                                tp = outp.tile([128, 128], f16, name="tp",
                                               tag="tp")
                                nc.sync.dma_start_transpose(
                                    tp[:], ots[h][:, t * 128:(t + 1) * 128]
                                )
                                src = tp
                            rcp = outp.tile([128, 1], f32, name="rcp", tag="rcp")
                            nc.vector.reciprocal(rcp[:], src[:, 64:65])
                            c0 = t * 128 + h * 64
                            if last and h == 0:
                                nc.scalar.mul(
                                    res[:, c0:c0 + 64], src[:, 0:64], rcp[:],
                                )
                            else:
                                nc.vector.tensor_scalar_mul(
                                    res[:, c0:c0 + 64], src[:, 0:64], rcp[:],
                                )
                    nc.sync.dma_start(
                        out[qc * 512:(qc + 1) * 512, :]
                        .rearrange("(t p) c -> p t c", t=4),
                        res[:].rearrange("p (t c) -> p t c", t=4),
                    )

                pop_cnt = {}    # (qc, h) -> key-tiles popped so far; start/
                                # stop by count since pop order may not be
                                # kc order once f8 pairs interleave with f16.

                def pop_ev(n, ps_o, keep=0):
                    # keep: leave at least this many entries in the fifo so
                    # pops trail exp completion and never head-block the
                    # in-order PE queue waiting on an unfinished et tile.
                    popped = 0
                    while len(ev_fifo) > keep and popped < n:
                        qc, kind, kc, h, et, off = ev_fifo[0]
                        # never emit a pop ahead of its V tile: a blocked
                        # matmul would head-block the in-order PE queue
                        need_kc = kc + (2 if kind == '8' else 1)
                        if need_kc + 1 > vq_done[0] and vq_done[0] < n_kc:
                            break
                        ev_fifo.pop(0)
                        if qc not in po_by_qc:
                            po_by_qc[qc] = [
                                ps_o.tile([128, 512], f32, name=f"po{h2}",
                                          tag=f"po{h2}")
                                for h2 in range(2)
                            ]
                        po = po_by_qc[qc]
                        cnt = pop_cnt.get((qc, h), 0)
                        if kind == '8':
                            # DoubleRow fp8: one pop covers keys of kc, kc+1
                            j, _ = f8_pair(kc)
                            base = j * 320 + h * 160
                            lhsT = (v1_8[:, base: base + 160]
                                    .rearrange("p (k c) -> p k c", k=2, c=80)
                                    [:, :, 0:65])
                            rhs = (et[:]
                                   .rearrange("p (k hh c) -> p k hh c",
                                              k=2, hh=2, c=512)[:, :, h, :])
                            nkc = 2
                        else:
                            lhsT = v1[:, kc * 130 + h * 65:
                                      kc * 130 + h * 65 + 65]
                            rhs = et[:, off:off + 512]
                            nkc = 1
                        pop_cnt[(qc, h)] = cnt + nkc
                        nc.tensor.matmul(
                            po[h][0:65, :],
                            lhsT=lhsT,
                            rhs=rhs,
                            start=(cnt == 0),
                            stop=(cnt + nkc == n_kc),
                            perf_mode=(mybir.MatmulPerfMode.DoubleRow
                                       if kind == '8' else None),
                        )
                        popped += 1
                        ev_left[qc] -= nkc
                        if ev_left[qc] == 0:
                            emit_norm(po_by_qc.pop(qc), qc)

                def fill_chunk(qc, batch, st_pool):
                    """S^T matmuls for one chunk into a st_pool ring tile.
                    Slices of the same kc (h0, h1) are emitted adjacently so
                    their K=64 matmuls co-run in different PE row groups."""
                    if qc not in ev_left:
                        ev_left[qc] = 2 * n_kc
                    qs = slice(qc * 512, (qc + 1) * 512)
                    w = len(batch) * 512
                    st_ps = st_pool.tile([128, w], f32, name="st_ps", tag="st")
                    for si, (kc, h) in enumerate(batch):
                        hp = slice(h * 64, (h + 1) * 64)
                        nc.tensor.matmul(
                            st_ps[:, si * 512:(si + 1) * 512],
                            lhsT=kt[hp, kc * 128:(kc + 1) * 128],
                            rhs=qt[hp, qs],
                            start=True,
                            stop=True,
                        )
                    return (qc, batch, st_ps)

                pair_tiles = {}   # (qc, j) -> [128, 2048] f8 pair et tile

                def exp_chunk(ctx, eng, fp8=False):
                    """exp of a filled chunk on ACT ('A') or DVE ('D').
                    fp8 A-chunks write half of a [128,2048] f8 pair tile;
                    pop entries for the pair are appended on its 2nd half."""
                    qc, batch, st_ps = ctx
                    w = len(batch) * 512
                    if fp8:
                        kc = batch[0][0]
                        j, k = f8_pair(kc)
                        if (qc, j) not in pair_tiles:
                            pair_tiles[(qc, j)] = etp.tile(
                                [128, 2048], f8, name="et8", tag="et")
                        et = pair_tiles[(qc, j)]
                        nc.scalar.activation(
                            et[:, k * 1024: k * 1024 + w], st_ps[:],
                            AF.Exp, scale=0.125,
                        )
                        if k == 1:
                            pair_tiles.pop((qc, j))
                            for h in range(2):
                                ev_fifo.append((qc, '8', kc - 1, h, et, 0))
                        return
                    et = etp.tile([128, w], f16, name="et", tag="et")
                    if eng == 'A':
                        nc.scalar.activation(et[:], st_ps[:], AF.Exp,
                                             scale=0.125)
                    else:
                        nc.vector._custom_dve(
                            EXP_1OP, out=et[:], in0=st_ps[:],
                            s0=_E1_C0, s1=_E1_C1, imm2=_E1_C2,
                        )
                    for si, (kc, h) in enumerate(batch):
                        ev_fifo.append((qc, '16', kc, h, et, si * 512))

                def emit_chunk(qc, batch, eng, st_pool):
                    exp_chunk(fill_chunk(qc, batch, st_pool), eng)

                def chunk_list(qc, sizes):
                    slices = [(kc, h) for kc in range(n_kc) for h in range(2)]
                    o, res = 0, []
                    for sz in sizes:
                        res.append(slices[o:o + sz])
                        o += sz
                    return res

                def sched_steady():
                    """32 kc-chunks (both heads, [128,1024]) in kc order;
                    engine per kc from KC_PATTERN (A-kc exp on ACT -> fp8
                    pair et; D-kc exp on DVE -> f16 et)."""
                    return [(KC_PATTERN[kc % len(KC_PATTERN)],
                             [(kc, 0), (kc, 1)]) for kc in range(n_kc)]

                # 22 chunks per qc: two 1024-wide starters, then 1536-wide.
                # The qc ends on a big chunk so the next qc's S^T matmuls
                # are always covered by >= 1.3us of exp time.
                SIZES = [2, 2] + [3] * 20

                # ---- qc0/qc1: attention + pipelined projections ----
                # No E@V pops here: the projection ring owns the two PSUM
                # banks that later hold the E@V accumulators; a deep fp16
                # ring buffers all prologue exp outputs instead.
                with (
                    tc.tile_pool(name="pproj", bufs=2, space="PSUM") as pproj,
                    tc.tile_pool(name="ps_pro", bufs=2, space="PSUM") as ps_pro,
                ):
                    pools['st'] = ps_pro
                    # ~3.4us of dummy matmuls while x block0 is in flight:
                    # trips the PE HAM clock-gate to 2.4GHz so the first
                    # real projections don't run at half clock
                    for w in range(8):
                        wp = ps_pro.tile([128, 512], f32, name="warm", tag="st")
                        nc.tensor.matmul(
                            wp[:], lhsT=ident[:], rhs=wk_sb[:, 0:512],
                            start=True, stop=True,
                        )
                    emit_kq(kt, wk_sb, bk_sb, 0, pproj)
                    emit_kq(qt, wq_sb, bq_sb, 0, pproj)
                    # per-chunk piece schedule: K blocks ahead of their S^T
                    # use and behind their x DMA; Q before its q-chunk
                    qc0_kq = {2 * b - 1: f"k{b}" for b in range(1, n_qc)}
                    qc0_kq.update({13 + 2 * b: f"q{b}" for b in range(1, 5)})
                    qc1_kq = {0: "q5", 2: "q6", 4: "q7"}
                    # ~5 chunks/qc on DVE relieve the ACT-bound prologue
                    PRO_DVE = {6, 10, 14, 18, 21}
                    for qc, kq, dbl in ((0, qc0_kq, ()), (1, qc1_kq, (16, 18, 20))):
                        for ci, batch in enumerate(chunk_list(qc, SIZES)):
                            emit_chunk(qc, batch,
                                       'D' if ci in PRO_DVE else 'A', ps_pro)
                            piece = kq.get(ci)
                            if piece is not None:
                                b = int(piece[1:])
                                if piece[0] == "k":
                                    emit_kq(kt, wk_sb, bk_sb, b, pproj)
                                else:
                                    emit_kq(qt, wq_sb, bq_sb, b, pproj)
                            elif not (qc == 0 and ci == 0) and vq_done[0] < n_kc:
                                emit_vq(pproj)
                                if ci in dbl and vq_done[0] < n_kc:
                                    emit_vq(pproj)

                # ---- qc2..qc7: decoupled ACT/DVE chunk streams ----
                # ps_o FIRST: it must overlay pproj's banks (free mid-qc1)
                # -- not ps_pro's, whose release needs every prologue exp,
                # which need et-ring slots, which need pops, which need
                # ps_o: a scheduling deadlock.
                #
                # One deep shared score ring (3 x [128,1024] = 6 banks):
                # each kc-chunk is consumed whole by ACT or DVE, fills lead
                # their exp by 2 chunks (~2us) so the in-order PE queue
                # never reaches a fill whose ring slot isn't already free,
                # and pops trail by POP_KEEP slices -- PE never waits.
                with (
                    tc.tile_pool(name="ps_o", bufs=1, space="PSUM") as ps_o,
                    tc.tile_pool(name="ps_a", bufs=3, space="PSUM") as ps_a,
                ):
                    pools['st'] = ps_a
                    plan = sched_steady()
                    chunks = [(qc, eng, batch)
                              for qc in range(2, n_qc)
                              for (eng, batch) in plan]
                    pend = []
                    for qc, eng, batch in chunks:
                        pend.append((fill_chunk(qc, batch, ps_a), eng))
                        if len(pend) >= 3:
                            ctx, e = pend.pop(0)
                            exp_chunk(ctx, e, fp8=(e == 'A'))
                        pop_ev(POP_Q_BY_QC.get(qc, POP_Q), ps_o,
                               keep=POP_KEEP)
                    for ctx, e in pend:
                        exp_chunk(ctx, e, fp8=(e == 'A'))
                    pop_ev(len(ev_fifo), ps_o)
    return nc


def _shard_inputs(x, Wq, bq, Wk, bk, Wv, bv):
    """Build the 8 per-core input maps from full inputs."""
    x = np.asarray(x, dtype=np.float32)
    in_maps = []
    for c in range(N_CORES):
        b, pair = c // 4, c % 4
        rows = slice(pair * 128, (pair + 1) * 128)
        wq_s = np.asarray(Wq)[rows, :].astype(np.float32)
        wk_s = np.asarray(Wk)[rows, :].astype(np.float32)
        wv_s = np.asarray(Wv)[rows, :].astype(np.float32)
        bq_s = np.asarray(bq)[rows].astype(np.float32)
        bk_s = np.asarray(bk)[rows].astype(np.float32)
        bv_s = np.asarray(bv)[rows].astype(np.float32)

        wvT = np.zeros((D_MODEL, 130), np.float32)
        wvT[:, 0:64] = wv_s[0:64].T
        wvT[:, 65:129] = wv_s[64:128].T
        wvT = wvT.reshape(4, 128, 130).transpose(1, 0, 2).reshape(128, 520)
        wq_im = wq_s.T.reshape(4, 128, 128).transpose(1, 0, 2).reshape(128, 512)
        wk_im = wk_s.T.reshape(4, 128, 128).transpose(1, 0, 2).reshape(128, 512)
        bvb = np.zeros((128, 130), np.float32)
        bvb[:, 0:64] = bv_s[0:64]
        bvb[:, 64] = 1.0
        bvb[:, 65:129] = bv_s[64:128]
        bvb[:, 129] = 1.0
        wq_im = np.concatenate([wq_im, bq_s.reshape(128, 1)], axis=1)
        wk_im = np.concatenate([wk_im, bk_s.reshape(128, 1)], axis=1)
        wvT = np.concatenate([wvT, bvb], axis=1)

        in_maps.append({
            "xT": np.ascontiguousarray(x[c // 4].T).astype(np.float16),
            "wqT": np.ascontiguousarray(wq_im).astype(np.float16),
            "wkT": np.ascontiguousarray(wk_im).astype(np.float16),
            "wvT": wvT.astype(np.float16),
        })
    return in_maps


def _gather(results):
    B, S, D = 2, S_FULL, D_MODEL
    out = np.empty((B, S, D), np.float32)
    for c in range(N_CORES):
        b, pair = c // 4, c % 4
        out[b, :, pair * 128:(pair + 1) * 128] = results[c]["out"]
    return out


def _install_profile_hook():
    """Provide antenv.axon_hooks (missing in this image) so that
    run_bass_kernel_spmd(trace=True) can capture NTFF profiles, using the
    same ctypes path trn_boot.py would have registered."""
    import sys, types, ctypes, contextlib

    if "antenv.axon_hooks" in sys.modules:
        return
    so_path = "/opt/axon/libaxon_pjrt.so"
    mod = types.ModuleType("antenv.axon_hooks")
    state = {"hook": None}
    mod.set_axon_ntff_profile_hook = lambda h: state.__setitem__("hook", h)
    mod.get_axon_ntff_profile_hook = lambda: state["hook"]
    sys.modules["antenv.axon_hooks"] = mod
    try:
        lib = ctypes.CDLL(so_path)
        if not hasattr(lib, "axon_start_nrt_profile"):
            return
        lib.axon_start_nrt_profile.argtypes = [
            ctypes.POINTER(ctypes.c_int64), ctypes.c_size_t]
        lib.axon_start_nrt_profile.restype = ctypes.c_int64
        lib.axon_stop_nrt_profile.argtypes = [ctypes.c_char_p]
        lib.axon_stop_nrt_profile.restype = ctypes.c_int64

        @contextlib.contextmanager
        def _hook(output_dir, device_ids):
            import jax
            jax.devices()
            if device_ids:
                ids = (ctypes.c_int64 * len(device_ids))(*device_ids)
                rc = lib.axon_start_nrt_profile(ids, len(device_ids))
            else:
                rc = lib.axon_start_nrt_profile(None, 0)
            if rc != 0:
                raise RuntimeError(f"axon_start_nrt_profile rc={rc}")
            try:
                yield
            finally:
                n = lib.axon_stop_nrt_profile(str(output_dir).encode())
                print(f"profile: {n} file(s) written to {output_dir}")

        state["hook"] = _hook
    except OSError:
        pass


def kernel(x, Wq, bq, Wk, bk, Wv, bv, trace=False):
    from concourse.bass_utils import run_bass_kernel_spmd

    if trace:
        _install_profile_hook()
    if "nc" not in _cached:
        nc = build_nc(S_FULL)
        nc.finalize()
        _cached["nc"] = nc
    nc = _cached["nc"]
    in_maps = _shard_inputs(x, Wq, bq, Wk, bk, Wv, bv)
    r = run_bass_kernel_spmd(nc, in_maps, list(range(N_CORES)), trace=trace)
    _cached["last_results"] = r
    return _gather(r.results)


# revision 52
# speedup vs baseline: 1.0489x; 1.0489x over previous
"""Multi-head attention (B=2, S=4096, D=512, H=8) on 8 NeuronCores.

Sharding: data-parallel on batch x head-pair-parallel.  Core c handles
batch b = c//4 and heads (2*(c%4), 2*(c%4)+1).  Each core computes its
[4096, 128] slice of the output; the host scatters inputs / gathers
outputs.

Per-core kernel (Bass/Tile), operands in fp16 (fp32 PSUM accumulate).
The exp over all 2*4096^2 scores is the hard bottleneck.  The scalar
(ACT) engine does it at ~0.87 ns/col; this kernel additionally routes a
fraction of the scores through the *vector* engine via a custom-DVE op
(EXP_ANT_1OP: deg-2 minimax poly of exp(s/128) + 4 chained squarings,
8 ALU stages, ~1.04 ns/col), so both engines burn down the softmax in
parallel:

  - prologue (qc0/qc1) is the baseline schedule: projections pipelined
    into the attention sweep, E@V deferred through a deep fp16 ring.
  - steady state (qc2+): per q-chunk the 64 (kc,h) scorx slices walk in
    order through two decoupled chunk streams: ACT chunks (2 slices,
    [128,1024], own 2x2-bank PSUM ring) and DVE chunks (1 slice,
    [128,512], own 2x1-bank ring), ~24/64 slices on DVE.  E@V pops and
    start/stop flags are unchanged since slice order is preserved.
  - PSUM: ACT ring 4 banks + DVE ring 2 + E@V accumulators 2 = 8.

Measured on the 8 axon trn2 cores: baseline 288.7 us (ACT ~258 us busy,
87%); this version targets ~200 us by offloading ~37% of exp to DVE.
"""

import numpy as np

N_CORES = 8
S_FULL = 4096
D_MODEL = 512
HEAD = 64

# steady-state schedule knobs.  Engine per kc: adjacent A-kc pairs run
# exp on ACT with fp8 et and are consumed by one DoubleRow (256-key)
# E@V pop per head; D-kc run exp on DVE (f16 et, plain f16 pops).
# 20 A (10 fp8 pairs) + 12 D balances ACT ~20.7us, DVE ~16.4us and cuts
# PE pops from 64 to 44 per qc.
KC_PATTERN = ('A', 'A', 'D') * 10 + ('D', 'D')
# Prologue engine override: these D-kc run their (f16) exp on ACT
# instead of DVE, which in qc0/qc1 also carries the projection adds.
PRO_ACT_D = {11, 23, 30}
POP_Q_BY_QC = {2: 4, 3: 4, 4: 4, 5: 4}  # pops per kc-chunk (default 3):
                                        # spread the deferred prologue
                                        # backlog over qc2-5
POP_Q = 3
POP_KEEP = 8          # min fifo depth: pops trail exp, never block PE

_cached = {}


def _register_exp_ops():
    """Register custom-DVE exp ops (rows 17+, free on TRN2).

    EXP_ANT_1OP: exp(0.125*s) ~= p(s)^16 with p = (s*c2 + c1)*s + c0,
    the relative-minimax deg-2 fit of exp(y) on y = s/128 in [-.215,.215]
    (raw scores |0.125*s| <= 3.44 observed <= 3.2).  4+4 = 8 ALU stages,
    1 elem/cycle/lane.  Worst rel err 1.1e-2 at the range edge, 4.7e-3
    for |z|<=2.5; end-to-end (softmax-normalized, f16 weights) the
    output delta is <2e-4 -- an order below the f16 noise floor budget.

    EXP_ANT_P1/P2: accurate 2-instruction chain (deg-3 Taylor of
    exp(s*2^-11) then 8 squarings), kept as fallback.
    """
    import concourse.dve_ops as dve_ops
    from concourse.dve_spec import Spec, Src0, C0, C1, C2, One, lower, sq, _has_src1
    from concourse.dve_uop import DveOpSpec

    have = {op.name: op for op in dve_ops.OPS}

    def reg(name, spec):
        if name in have:
            return have[name]
        row = dve_ops._CUSTOM_DVE_ROW_BASE + len(dve_ops.OPS)
        dve_ops._SUB_OPCODE_FOR_NAME[name] = row
        uops = lower(spec, ver="v3")
        sha3 = DveOpSpec(
            name=name, opcode=row, uops=uops, rd1_en=_has_src1(spec)
        ).sha("v3")
        op = dve_ops.DveOp(name, spec, subdim=False, uops_sha={"v3": sha3})
        dve_ops.OPS.append(op)
        dve_ops.CUSTOM_DVE_SPECS[name] = spec
        return op

    x = (Src0 * C2 + C1) * Src0 + C0
    for _ in range(4):
        x = sq(x)
    spec_1op = Spec(
        body=x,
        reference=lambda in0, in1, s0, s1, imm2: (
            ((in0.astype(np.float32) * np.float32(imm2) + np.float32(s1))
             * in0 + np.float32(s0)).astype(np.float32) ** 16
        ).astype(np.float32),
    )

    body1 = ((Src0 * C2 + C1) * Src0 + C0) * Src0 + One
    spec_p1 = Spec(
        body=body1,
        reference=lambda in0, in1, s0, s1, imm2: (
            ((in0.astype(np.float32) * np.float32(imm2) + np.float32(s1))
             * in0 + np.float32(s0)) * in0 + np.float32(1.0)
        ).astype(np.float32),
    )
    y = Src0
    for _ in range(8):
        y = sq(y)
    spec_p2 = Spec(
        body=y,
        reference=lambda in0, in1, s0, s1, imm2: (
            in0.astype(np.float32) ** 256
        ).astype(np.float32),
    )
    return (
        reg("EXP_ANT_1OP", spec_1op),
        reg("EXP_ANT_P1", spec_p1),
        reg("EXP_ANT_P2", spec_p2),
    )


# minimax deg-2 fit of exp(y), y = s/128, |y| <= 3.44/16; s-domain coeffs
_E1_C0 = 1.0000528961258521
_E1_C1 = 0.007848291635150783
_E1_C2 = 3.0376679783362523e-05


def build_nc(S=S_FULL):
    import concourse.bass as bass
    from concourse import bacc
    import concourse.mybir as mybir
    import concourse.tile as tile
    from concourse.masks import make_identity
    f32 = mybir.dt.float32
    f16 = mybir.dt.float16
    f8 = mybir.dt.float8e4
    AF = mybir.ActivationFunctionType

    D = D_MODEL
    n_qc = S // 512     # 512-wide query chunks
    n_kc = S // 128     # 128-wide key tiles
    n_dc = D // 128     # 128-wide contraction chunks of D

    EXP_1OP, EXP_P1, EXP_P2 = _register_exp_ops()

    nc = bacc.Bacc()

    xT = nc.dram_tensor("xT", [D, S], f16, kind="ExternalInput")
    # weights arrive in SBUF-image layout with the bias as a trailing
    # column (f16 bias rounding is far below fp16 operand noise): one
    # contiguous DMA each
    wqT = nc.dram_tensor("wqT", [128, n_dc * 128 + 1], f16, kind="ExternalInput")
    wkT = nc.dram_tensor("wkT", [128, n_dc * 128 + 1], f16, kind="ExternalInput")
    wvT = nc.dram_tensor("wvT", [128, n_dc * 130 + 130], f16,
                         kind="ExternalInput")
    out = nc.dram_tensor("out", [S, 128], f32, kind="ExternalOutput")

    with tile.TileContext(nc) as tc:
        with (
            tc.tile_pool(name="consts", bufs=1) as consts,
            tc.tile_pool(name="persist", bufs=1) as persist,
        ):
            ident = consts.tile([128, 128], f16, name="ident")
            tiny = consts.tile([128, 8], f32, name="tiny")
            tiny_o = consts.tile([128, 8], f16, name="tiny_o")
            wq_sb = consts.tile([128, n_dc * 128 + 1], f16, name="wq_sb")
            wk_sb = consts.tile([128, n_dc * 128 + 1], f16, name="wk_sb")
            wv_sb = consts.tile([128, n_dc * 130 + 130], f16, name="wv_sb")
            bq_sb = consts.tile([128, 1], f32, name="bq_sb")
            bk_sb = consts.tile([128, 1], f32, name="bk_sb")
            bvb_sb = consts.tile([128, 130], f32, name="bvb_sb")
            xt = persist.tile([128, n_dc * S], f16, name="xt")

            def xs(dc, sl):
                return xt[:, dc * S + sl.start: dc * S + sl.stop]
            qt = persist.tile([128, S], f16, name="qt")
            kt = persist.tile([128, S], f16, name="kt")
            # V1[kc*130 + h*65 : +65] = [V_h | ones] per key tile kc.
            v1 = persist.tile([128, n_kc * 130], f16, name="v1")
            # fp8 shadow of V for the DoubleRow pair-pops, pair-major:
            # col = j*320 + h*160 + k*80 + d for f8 pair j, half k, dim d<65.
            # The 80-col k-slots satisfy the dual-fp8 Ldweights ISA rule
            # (free-dim steps even and 16B-aligned).  Only A-kc live here.
            n_p8 = sum(1 for e in KC_PATTERN if e == 'A') * (n_kc // len(KC_PATTERN)) // 2
            # +160 pad so the k=1 shadow write can view a full 320-col window
            v1_8 = persist.tile([128, n_p8 * 320 + 160], f8, name="v1_8")

            # ACT table preload: dummy exp at t=0 hides the ~2.7us load.
            nc.vector.memset(tiny[:], 0.0)
            nc.scalar.activation(tiny_o[:], tiny[:], AF.Exp, scale=0.125)
            make_identity(nc, ident)

            # DMAs.  Issue order IS priority: each dma_start costs
            # 0.6-1.4us of Sync issue time and the DMA engines round-robin
            # fairly over everything outstanding, so: tiny weights first,
            # then x block0 as a single 3D transfer, then the later x
            # pieces (xB is also held behind xA by a deliberate 1-column
            # WAW overlap so block0+xA are never starved).
            x_src = xT[:, :].rearrange("(dc p) s -> p dc s", dc=n_dc)
            x_dst = xt[:].rearrange("p (dc s) -> p dc s", s=S)
            nc.sync.dma_start(wk_sb[:], wkT[:, :])
            nc.sync.dma_start(wq_sb[:], wqT[:, :])
            nc.sync.dma_start(x_dst[:, :, 0:512], x_src[:, :, 0:512])
            nc.sync.dma_start(x_dst[:, :, 512:1024], x_src[:, :, 512:1024])
            nc.sync.dma_start(wv_sb[:], wvT[:, :])
            nc.sync.dma_start(x_dst[:, :, 1024:2049], x_src[:, :, 1024:2049])
            nc.sync.dma_start(x_dst[:, :, 2048:S], x_src[:, :, 2048:S])
            # biases ride in the weight images as f16; widen to f32 once
            nc.vector.tensor_copy(bk_sb[:], wk_sb[:, n_dc * 128: n_dc * 128 + 1])
            nc.vector.tensor_copy(bq_sb[:], wq_sb[:, n_dc * 128: n_dc * 128 + 1])
            nc.vector.tensor_copy(bvb_sb[:], wv_sb[:, n_dc * 130: n_dc * 130 + 130])

            with (
                # 48 2KB slots: holds every deferred prologue et (44) plus
                # steady in-flight -- no prologue exp ever waits on a pop
                tc.tile_pool(name="etp", bufs=48) as etp,
                tc.tile_pool(name="outp", bufs=2) as outp,
            ):
                pools = {}  # 'st': pool for emit_norm's PE transposes

                # ---------- projection pieces ----------
                def emit_kq(dst, w_sb, b_sb, b, pool):
                    cs = slice(b * 512, (b + 1) * 512)
                    p = pool.tile([128, 512], f32, name="pp", tag="pp")
                    for dc in range(n_dc):
                        nc.tensor.matmul(
                            p[:],
                            lhsT=w_sb[:, dc * 128:(dc + 1) * 128],
                            rhs=xs(dc, cs),
                            start=(dc == 0),
                            stop=(dc == n_dc - 1),
                        )
                    nc.vector.tensor_scalar_add(dst[:, cs], p[:], b_sb[:])

                vq_done = [0]

                def is_f8_kc(kc):
                    return KC_PATTERN[kc % len(KC_PATTERN)] == 'A'

                def f8_pair(kc):
                    """(pair index j, half k) for an f8 kc under KC_PATTERN."""
                    per = len(KC_PATTERN)
                    n_a_per = sum(1 for e in KC_PATTERN if e == 'A')
                    idx = (kc // per) * n_a_per + sum(
                        1 for e in KC_PATTERN[:kc % per] if e == 'A')
                    return idx // 2, idx % 2

                def emit_vq(pool):
                    # V projection for the next 128-token tile.
                    st_ = vq_done[0]
                    ss = slice(st_ * 128, (st_ + 1) * 128)
                    p = pool.tile([128, 512], f32, name="pp", tag="pp")
                    for dc in range(n_dc):
                        nc.tensor.matmul(
                            p[:, 0:130],
                            lhsT=xs(dc, ss),
                            rhs=wv_sb[:, dc * 130:(dc + 1) * 130],
                            start=(dc == 0),
                            stop=(dc == n_dc - 1),
                        )
                    nc.vector.tensor_add(
                        v1[:, st_ * 130:(st_ + 1) * 130], p[:, 0:130], bvb_sb[:]
                    )
                    if is_f8_kc(st_):
                        # fp8 shadow for DoubleRow pops; col layout
                        # j*320 + h*160 + k*80 + d -> h-strided view
                        j, k = f8_pair(st_)
                        base = j * 320 + k * 80
                        dst = (v1_8[:, base: base + 320]
                               .rearrange("p (h c) -> p h c", h=2, c=160)
                               [:, :, 0:65])
                        src = (p[:, 0:130]
                               .rearrange("p (h c) -> p h c", h=2, c=65))
                        bsrc = (bvb_sb[:, 0:130]
                                .rearrange("p (h c) -> p h c", h=2, c=65))
                        nc.vector.tensor_add(dst, src, bsrc)
                    vq_done[0] += 1

                # ---------- attention ----------
                ev_fifo = []        # (qc, kc, h, et_tile, col_off)
                ev_left = {}        # qc -> slices not yet popped
                po_by_qc = {}

                def emit_norm(po, qc):
                    # res[:, t*128+h*64 : +64] = head h of output rows
                    # qc*512 + t*128 + [0:128); shipped as one 3D DMA
                    res = outp.tile([128, 512], f32, name="res", tag="res")
                    last = qc == n_qc - 1
                    ots = []
                    for h in range(2):
                        ot = outp.tile([128, 512], f16, name="ot", tag="ot")
                        if last and h == 0:
                            nc.scalar.copy(ot[:], po[h][:])
                        else:
                            nc.vector.tensor_copy(ot[:], po[h][:])
                        ots.append(ot)
                    for t in range(4):
                        for h in range(2):
                            if last:
                                pt = pools['st'].tile([128, 65], f16, name="pt",
                                                      tag="st")
                                nc.tensor.transpose(
                                    pt[:],
                                    ots[h][0:65, t * 128:(t + 1) * 128],
                                    ident[0:65, 0:65],
                                )
                                src = pt
                            else:
                                tp = outp.tile([128, 128], f16, name="tp",
                                               tag="tp")
                                nc.sync.dma_start_transpose(
                                    tp[:], ots[h][:, t * 128:(t + 1) * 128]
                                )
                                src = tp
                            rcp = outp.tile([128, 1], f32, name="rcp", tag="rcp")
                            nc.vector.reciprocal(rcp[:], src[:, 64:65])
                            c0 = t * 128 + h * 64
                            if last and h == 0:
                                nc.scalar.mul(
                                    res[:, c0:c0 + 64], src[:, 0:64], rcp[:],
                                )
                            else:
                                nc.vector.tensor_scalar_mul(
                                    res[:, c0:c0 + 64], src[:, 0:64], rcp[:],
                                )
                    nc.sync.dma_start(
                        out[qc * 512:(qc + 1) * 512, :]
                        .rearrange("(t p) c -> p t c", t=4),
                        res[:].rearrange("p (t c) -> p t c", t=4),
                    )

                pop_cnt = {}    # (qc, h) -> key-tiles popped so far; start/
                                # stop by count since pop order may not be
                                # kc order once f8 pairs interleave with f16.

                def pop_ev(n, ps_o, keep=0):
                    # keep: leave at least this many entries in the fifo so
                    # pops trail exp completion and never head-block the
                    # in-order PE queue waiting on an unfinished et tile.
                    popped = 0
                    while len(ev_fifo) > keep and popped < n:
                        qc, kind, kc, h, et, off = ev_fifo[0]
                        # never emit a pop ahead of its V tile: a blocked
                        # matmul would head-block the in-order PE queue
                        need_kc = kc + (2 if kind == '8' else 1)
                        if need_kc + 1 > vq_done[0] and vq_done[0] < n_kc:
                            break
                        ev_fifo.pop(0)
                        if qc not in po_by_qc:
                            po_by_qc[qc] = [
                                ps_o.tile([128, 512], f32, name=f"po{h2}",
                                          tag=f"po{h2}")
                                for h2 in range(2)
                            ]
                        po = po_by_qc[qc]
                        cnt = pop_cnt.get((qc, h), 0)
                        if kind == '8':
                            # DoubleRow fp8: one pop covers keys of kc, kc+1
                            j, _ = f8_pair(kc)
                            base = j * 320 + h * 160
                            lhsT = (v1_8[:, base: base + 160]
                                    .rearrange("p (k c) -> p k c", k=2, c=80)
                                    [:, :, 0:65])
                            rhs = (et[:]
                                   .rearrange("p (k hh c) -> p k hh c",
                                              k=2, hh=2, c=512)[:, :, h, :])
                            nkc = 2
                        else:
                            lhsT = v1[:, kc * 130 + h * 65:
                                      kc * 130 + h * 65 + 65]
                            rhs = et[:, off:off + 512]
                            nkc = 1
                        pop_cnt[(qc, h)] = cnt + nkc
                        nc.tensor.matmul(
                            po[h][0:65, :],
                            lhsT=lhsT,
                            rhs=rhs,
                            start=(cnt == 0),
                            stop=(cnt + nkc == n_kc),
                            perf_mode=(mybir.MatmulPerfMode.DoubleRow
                                       if kind == '8' else None),
                        )
                        popped += 1
                        ev_left[qc] -= nkc
                        if ev_left[qc] == 0:
                            emit_norm(po_by_qc.pop(qc), qc)

                def fill_chunk(qc, batch, st_pool):
                    """S^T matmuls for one chunk into a st_pool ring tile.
                    Slices of the same kc (h0, h1) are emitted adjacently so
                    their K=64 matmuls co-run in different PE row groups."""
                    if qc not in ev_left:
                        ev_left[qc] = 2 * n_kc
                    qs = slice(qc * 512, (qc + 1) * 512)
                    w = len(batch) * 512
                    st_ps = st_pool.tile([128, w], f32, name="st_ps", tag="st")
                    for si, (kc, h) in enumerate(batch):
                        hp = slice(h * 64, (h + 1) * 64)
                        nc.tensor.matmul(
                            st_ps[:, si * 512:(si + 1) * 512],
                            lhsT=kt[hp, kc * 128:(kc + 1) * 128],
                            rhs=qt[hp, qs],
                            start=True,
                            stop=True,
                        )
                    return (qc, batch, st_ps)

                pair_tiles = {}   # (qc, j) -> [128, 2048] f8 pair et tile

                def exp_chunk(ctx, eng, fp8=False):
                    """exp of a filled chunk on ACT ('A') or DVE ('D').
                    fp8 A-chunks write half of a [128,2048] f8 pair tile;
                    pop entries for the pair are appended on its 2nd half."""
                    qc, batch, st_ps = ctx
                    w = len(batch) * 512
                    if fp8:
                        kc = batch[0][0]
                        j, k = f8_pair(kc)
                        if (qc, j) not in pair_tiles:
                            pair_tiles[(qc, j)] = etp.tile(
                                [128, 2048], f8, name="et8", tag="et")
                        et = pair_tiles[(qc, j)]
                        nc.scalar.activation(
                            et[:, k * 1024: k * 1024 + w], st_ps[:],
                            AF.Exp, scale=0.125,
                        )
                        if k == 1:
                            pair_tiles.pop((qc, j))
                            for h in range(2):
                                ev_fifo.append((qc, '8', kc - 1, h, et, 0))
                        return
                    et = etp.tile([128, w], f16, name="et", tag="et")
                    if eng == 'A':
                        nc.scalar.activation(et[:], st_ps[:], AF.Exp,
                                             scale=0.125)
                    else:
                        nc.vector._custom_dve(
                            EXP_1OP, out=et[:], in0=st_ps[:],
                            s0=_E1_C0, s1=_E1_C1, imm2=_E1_C2,
                        )
                    for si, (kc, h) in enumerate(batch):
                        ev_fifo.append((qc, '16', kc, h, et, si * 512))

                def emit_chunk(qc, batch, eng, st_pool):
                    exp_chunk(fill_chunk(qc, batch, st_pool), eng)

                def sched(qc):
                    """32 kc-chunks (both heads, [128,1024]) in kc order;
                    engine per kc from KC_PATTERN (A-kc exp on ACT -> fp8
                    pair et; D-kc exp on DVE -> f16 et).  In the prologue
                    a few D-kc shift to ACT (f16): DVE also carries the
                    projection bias adds there."""
                    res = []
                    for kc in range(n_kc):
                        eng = KC_PATTERN[kc % len(KC_PATTERN)]
                        f8c = eng == 'A'
                        if qc < 2 and kc in PRO_ACT_D:
                            eng = 'A'
                        res.append((eng, f8c, [(kc, 0), (kc, 1)]))
                    return res

                # ---- qc0/qc1: attention + pipelined projections ----
                # No E@V pops here: the projection ring owns the two PSUM
                # banks that later hold the E@V accumulators; the 48-slot
                # et ring buffers all prologue exp outputs instead.
                with (
                    tc.tile_pool(name="pproj", bufs=2, space="PSUM") as pproj,
                    tc.tile_pool(name="ps_pro", bufs=3, space="PSUM") as ps_pro,
                ):
                    pools['st'] = ps_pro
                    # ~3.4us of dummy matmuls while x block0 is in flight:
                    # trips the PE HAM clock-gate to 2.4GHz so the first
                    # real projections don't run at half clock
                    for w in range(8):
                        wp = ps_pro.tile([128, 512], f32, name="warm", tag="st")
                        nc.tensor.matmul(
                            wp[:], lhsT=ident[:], rhs=wk_sb[:, 0:512],
                            start=True, stop=True,
                        )
                    emit_kq(kt, wk_sb, bk_sb, 0, pproj)
                    emit_kq(qt, wq_sb, bq_sb, 0, pproj)
                    # per-chunk piece schedule: K blocks ahead of their S^T
                    # use (fills of kc 4b need K_b) and behind their x DMA
                    # (timing matches the proven 22-chunk schedule, scaled
                    # 22->32); Q_b lands before its q-chunk.
                    qc0_kq = {1: "k1", 4: "k2", 7: "k3", 10: "k4", 13: "k5",
                              16: "k6", 19: "k7", 22: "q1", 25: "q2",
                              28: "q3", 31: "q4"}
                    qc1_kq = {0: "q5", 3: "q6", 6: "q7"}
                    pend = []
                    for qc, kq in ((0, qc0_kq), (1, qc1_kq)):
                        for ci, (eng, f8c, batch) in enumerate(sched(qc)):
                            pend.append(
                                (fill_chunk(qc, batch, ps_pro), eng, f8c))
                            if len(pend) >= 3:
                                ctx, e, f8e = pend.pop(0)
                                exp_chunk(ctx, e, fp8=f8e)
                            piece = kq.get(ci)
                            if piece is not None:
                                b = int(piece[1:])
                                if piece[0] == "k":
                                    emit_kq(kt, wk_sb, bk_sb, b, pproj)
                                else:
                                    emit_kq(qt, wq_sb, bq_sb, b, pproj)
                            elif not (qc == 0 and ci == 0) and vq_done[0] < n_kc:
                                emit_vq(pproj)
                    for ctx, e, f8e in pend:
                        exp_chunk(ctx, e, fp8=f8e)
                    pend = []

                # ---- qc2..qc7: decoupled ACT/DVE chunk streams ----
                # ps_o FIRST: it must overlay pproj's banks (free mid-qc1)
                # -- not ps_pro's, whose release needs every prologue exp,
                # which need et-ring slots, which need pops, which need
                # ps_o: a scheduling deadlock.
                #
                # One deep shared score ring (3 x [128,1024] = 6 banks):
                # each kc-chunk is consumed whole by ACT or DVE, fills lead
                # their exp by 2 chunks (~2us) so the in-order PE queue
                # never reaches a fill whose ring slot isn't already free,
                # and pops trail by POP_KEEP slices -- PE never waits.
                with (
                    tc.tile_pool(name="ps_o", bufs=1, space="PSUM") as ps_o,
                    tc.tile_pool(name="ps_a", bufs=3, space="PSUM") as ps_a,
                ):
                    pools['st'] = ps_a
                    chunks = [(qc, eng, f8c, batch)
                              for qc in range(2, n_qc)
                              for (eng, f8c, batch) in sched(qc)]
                    pend = []
                    for qc, eng, f8c, batch in chunks:
                        pend.append((fill_chunk(qc, batch, ps_a), eng, f8c))
                        if len(pend) >= 3:
                            ctx, e, f8e = pend.pop(0)
                            exp_chunk(ctx, e, fp8=f8e)
                        pop_ev(POP_Q_BY_QC.get(qc, POP_Q), ps_o,
                               keep=POP_KEEP)
                    for ctx, e, f8e in pend:
                        exp_chunk(ctx, e, fp8=f8e)
                    pop_ev(len(ev_fifo), ps_o)
    return nc


def _shard_inputs(x, Wq, bq, Wk, bk, Wv, bv):
    """Build the 8 per-core input maps from full inputs."""
    x = np.asarray(x, dtype=np.float32)
    in_maps = []
    for c in range(N_CORES):
        b, pair = c // 4, c % 4
        rows = slice(pair * 128, (pair + 1) * 128)
        wq_s = np.asarray(Wq)[rows, :].astype(np.float32)
        wk_s = np.asarray(Wk)[rows, :].astype(np.float32)
        wv_s = np.asarray(Wv)[rows, :].astype(np.float32)
        bq_s = np.asarray(bq)[rows].astype(np.float32)
        bk_s = np.asarray(bk)[rows].astype(np.float32)
        bv_s = np.asarray(bv)[rows].astype(np.float32)

        wvT = np.zeros((D_MODEL, 130), np.float32)
        wvT[:, 0:64] = wv_s[0:64].T
        wvT[:, 65:129] = wv_s[64:128].T
        wvT = wvT.reshape(4, 128, 130).transpose(1, 0, 2).reshape(128, 520)
        wq_im = wq_s.T.reshape(4, 128, 128).transpose(1, 0, 2).reshape(128, 512)
        wk_im = wk_s.T.reshape(4, 128, 128).transpose(1, 0, 2).reshape(128, 512)
        bvb = np.zeros((128, 130), np.float32)
        bvb[:, 0:64] = bv_s[0:64]
        bvb[:, 64] = 1.0
        bvb[:, 65:129] = bv_s[64:128]
        bvb[:, 129] = 1.0
        wq_im = np.concatenate([wq_im, bq_s.reshape(128, 1)], axis=1)
        wk_im = np.concatenate([wk_im, bk_s.reshape(128, 1)], axis=1)
        wvT = np.concatenate([wvT, bvb], axis=1)

        in_maps.append({
            "xT": np.ascontiguousarray(x[c // 4].T).astype(np.float16),
            "wqT": np.ascontiguousarray(wq_im).astype(np.float16),
            "wkT": np.ascontiguousarray(wk_im).astype(np.float16),
            "wvT": wvT.astype(np.float16),
        })
    return in_maps


def _gather(results):
    B, S, D = 2, S_FULL, D_MODEL
    out = np.empty((B, S, D), np.float32)
    for c in range(N_CORES):
        b, pair = c // 4, c % 4
        out[b, :, pair * 128:(pair + 1) * 128] = results[c]["out"]
    return out


def _install_profile_hook():
    """Provide antenv.axon_hooks (missing in this image) so that
    run_bass_kernel_spmd(trace=True) can capture NTFF profiles, using the
    same ctypes path trn_boot.py would have registered."""
    import sys, types, ctypes, contextlib

    if "antenv.axon_hooks" in sys.modules:
        return
    so_path = "/opt/axon/libaxon_pjrt.so"
    mod = types.ModuleType("antenv.axon_hooks")
    state = {"hook": None}
    mod.set_axon_ntff_profile_hook = lambda h: state.__setitem__("hook", h)
    mod.get_axon_ntff_profile_hook = lambda: state["hook"]
    sys.modules["antenv.axon_hooks"] = mod
    try:
        lib = ctypes.CDLL(so_path)
        if not hasattr(lib, "axon_start_nrt_profile"):
            return
        lib.axon_start_nrt_profile.argtypes = [
            ctypes.POINTER(ctypes.c_int64), ctypes.c_size_t]
        lib.axon_start_nrt_profile.restype = ctypes.c_int64
        lib.axon_stop_nrt_profile.argtypes = [ctypes.c_char_p]
        lib.axon_stop_nrt_profile.restype = ctypes.c_int64

        @contextlib.contextmanager
        def _hook(output_dir, device_ids):
            import jax
            jax.devices()
            if device_ids:
                ids = (ctypes.c_int64 * len(device_ids))(*device_ids)
                rc = lib.axon_start_nrt_profile(ids, len(device_ids))
            else:
                rc = lib.axon_start_nrt_profile(None, 0)
            if rc != 0:
                raise RuntimeError(f"axon_start_nrt_profile rc={rc}")
            try:
                yield
            finally:
                n = lib.axon_stop_nrt_profile(str(output_dir).encode())
                print(f"profile: {n} file(s) written to {output_dir}")

        state["hook"] = _hook
    except OSError:
        pass


def kernel(x, Wq, bq, Wk, bk, Wv, bv, trace=False):
    from concourse.bass_utils import run_bass_kernel_spmd

    if trace:
        _install_profile_hook()
    if "nc" not in _cached:
        nc = build_nc(S_FULL)
        nc.finalize()
        _cached["nc"] = nc
    nc = _cached["nc"]
    in_maps = _shard_inputs(x, Wq, bq, Wk, bk, Wv, bv)
    r = run_bass_kernel_spmd(nc, in_maps, list(range(N_CORES)), trace=trace)
    _cached["last_results"] = r
    return _gather(r.results)


# revision 54
# speedup vs baseline: 1.0590x; 1.0096x over previous
"""Multi-head attention (B=2, S=4096, D=512, H=8) on 8 NeuronCores.

Sharding: data-parallel on batch x head-pair-parallel.  Core c handles
batch b = c//4 and heads (2*(c%4), 2*(c%4)+1).  Each core computes its
[4096, 128] slice of the output; the host scatters inputs / gathers
outputs.

Per-core kernel (Bass/Tile), operands in fp16 (fp32 PSUM accumulate).
The exp over all 2*4096^2 scores is the hard bottleneck.  The scalar
(ACT) engine does it at ~0.87 ns/col; this kernel additionally routes a
fraction of the scores through the *vector* engine via a custom-DVE op
(EXP_ANT_1OP: deg-2 minimax poly of exp(s/128) + 4 chained squarings,
8 ALU stages, ~1.04 ns/col), so both engines burn down the softmax in
parallel:

  - prologue (qc0/qc1) is the baseline schedule: projections pipelined
    into the attention sweep, E@V deferred through a deep fp16 ring.
  - steady state (qc2+): per q-chunk the 64 (kc,h) scorx slices walk in
    order through two decoupled chunk streams: ACT chunks (2 slices,
    [128,1024], own 2x2-bank PSUM ring) and DVE chunks (1 slice,
    [128,512], own 2x1-bank ring), ~24/64 slices on DVE.  E@V pops and
    start/stop flags are unchanged since slice order is preserved.
  - PSUM: ACT ring 4 banks + DVE ring 2 + E@V accumulators 2 = 8.

Measured on the 8 axon trn2 cores: baseline 288.7 us (ACT ~258 us busy,
87%); this version targets ~200 us by offloading ~37% of exp to DVE.
"""

import numpy as np

N_CORES = 8
S_FULL = 4096
D_MODEL = 512
HEAD = 64

# steady-state schedule knobs.  Engine per kc: adjacent A-kc pairs run
# exp on ACT with fp8 et and are consumed by one DoubleRow (256-key)
# E@V pop per head; D-kc run exp on DVE (f16 et, plain f16 pops).
# 20 A (10 fp8 pairs) + 12 D balances ACT ~20.7us, DVE ~16.4us and cuts
# PE pops from 64 to 44 per qc.
KC_PATTERN = ('A', 'A', 'D') * 10 + ('D', 'D')
# Prologue engine override: these D-kc run their (f16) exp on ACT
# instead of DVE, which in qc0/qc1 also carries the projection adds.
PRO_ACT_D = {11, 23, 30}
POP_Q = 3
# Pops trail the exps by one full q-chunk (44 fifo entries): every
# steady qc then pays a flat one-qc pop bill on PE -- the deferred
# prologue backlog amortizes into the pipeline instead of crushing
# qc2-4.  The last qc drains aggressively to keep the tail short.
POP_KEEP = 44
POP_KEEP_LAST = 8

_cached = {}


def _register_exp_ops():
    """Register custom-DVE exp ops (rows 17+, free on TRN2).

    EXP_ANT_1OP: exp(0.125*s) ~= p(s)^16 with p = (s*c2 + c1)*s + c0,
    the relative-minimax deg-2 fit of exp(y) on y = s/128 in [-.215,.215]
    (raw scores |0.125*s| <= 3.44 observed <= 3.2).  4+4 = 8 ALU stages,
    1 elem/cycle/lane.  Worst rel err 1.1e-2 at the range edge, 4.7e-3
    for |z|<=2.5; end-to-end (softmax-normalized, f16 weights) the
    output delta is <2e-4 -- an order below the f16 noise floor budget.

    EXP_ANT_P1/P2: accurate 2-instruction chain (deg-3 Taylor of
    exp(s*2^-11) then 8 squarings), kept as fallback.
    """
    import concourse.dve_ops as dve_ops
    from concourse.dve_spec import Spec, Src0, C0, C1, C2, One, lower, sq, _has_src1
    from concourse.dve_uop import DveOpSpec

    have = {op.name: op for op in dve_ops.OPS}

    def reg(name, spec):
        if name in have:
            return have[name]
        row = dve_ops._CUSTOM_DVE_ROW_BASE + len(dve_ops.OPS)
        dve_ops._SUB_OPCODE_FOR_NAME[name] = row
        uops = lower(spec, ver="v3")
        sha3 = DveOpSpec(
            name=name, opcode=row, uops=uops, rd1_en=_has_src1(spec)
        ).sha("v3")
        op = dve_ops.DveOp(name, spec, subdim=False, uops_sha={"v3": sha3})
        dve_ops.OPS.append(op)
        dve_ops.CUSTOM_DVE_SPECS[name] = spec
        return op

    x = (Src0 * C2 + C1) * Src0 + C0
    for _ in range(4):
        x = sq(x)
    spec_1op = Spec(
        body=x,
        reference=lambda in0, in1, s0, s1, imm2: (
            ((in0.astype(np.float32) * np.float32(imm2) + np.float32(s1))
             * in0 + np.float32(s0)).astype(np.float32) ** 16
        ).astype(np.float32),
    )

    body1 = ((Src0 * C2 + C1) * Src0 + C0) * Src0 + One
    spec_p1 = Spec(
        body=body1,
        reference=lambda in0, in1, s0, s1, imm2: (
            ((in0.astype(np.float32) * np.float32(imm2) + np.float32(s1))
             * in0 + np.float32(s0)) * in0 + np.float32(1.0)
        ).astype(np.float32),
    )
    y = Src0
    for _ in range(8):
        y = sq(y)
    spec_p2 = Spec(
        body=y,
        reference=lambda in0, in1, s0, s1, imm2: (
            in0.astype(np.float32) ** 256
        ).astype(np.float32),
    )
    return (
        reg("EXP_ANT_1OP", spec_1op),
        reg("EXP_ANT_P1", spec_p1),
        reg("EXP_ANT_P2", spec_p2),
    )


# minimax deg-2 fit of exp(y), y = s/128, |y| <= 3.44/16; s-domain coeffs
_E1_C0 = 1.0000528961258521
_E1_C1 = 0.007848291635150783
_E1_C2 = 3.0376679783362523e-05


def build_nc(S=S_FULL):
    import concourse.bass as bass
    from concourse import bacc
    import concourse.mybir as mybir
    import concourse.tile as tile
    from concourse.masks import make_identity
    f32 = mybir.dt.float32
    f16 = mybir.dt.float16
    f8 = mybir.dt.float8e4
    AF = mybir.ActivationFunctionType

    D = D_MODEL
    n_qc = S // 512     # 512-wide query chunks
    n_kc = S // 128     # 128-wide key tiles
    n_dc = D // 128     # 128-wide contraction chunks of D

    EXP_1OP, EXP_P1, EXP_P2 = _register_exp_ops()

    nc = bacc.Bacc()

    xT = nc.dram_tensor("xT", [D, S], f16, kind="ExternalInput")
    # weights arrive in SBUF-image layout with the bias as a trailing
    # column (f16 bias rounding is far below fp16 operand noise): one
    # contiguous DMA each
    wqT = nc.dram_tensor("wqT", [128, n_dc * 128 + 1], f16, kind="ExternalInput")
    wkT = nc.dram_tensor("wkT", [128, n_dc * 128 + 1], f16, kind="ExternalInput")
    wvT = nc.dram_tensor("wvT", [128, n_dc * 130 + 130], f16,
                         kind="ExternalInput")
    out = nc.dram_tensor("out", [S, 128], f32, kind="ExternalOutput")

    with tile.TileContext(nc) as tc:
        with (
            tc.tile_pool(name="consts", bufs=1) as consts,
            tc.tile_pool(name="persist", bufs=1) as persist,
        ):
            ident = consts.tile([128, 128], f16, name="ident")
            tiny = consts.tile([128, 8], f32, name="tiny")
            tiny_o = consts.tile([128, 8], f16, name="tiny_o")
            wq_sb = consts.tile([128, n_dc * 128 + 1], f16, name="wq_sb")
            wk_sb = consts.tile([128, n_dc * 128 + 1], f16, name="wk_sb")
            wv_sb = consts.tile([128, n_dc * 130 + 130], f16, name="wv_sb")
            bq_sb = consts.tile([128, 1], f32, name="bq_sb")
            bk_sb = consts.tile([128, 1], f32, name="bk_sb")
            bvb_sb = consts.tile([128, 130], f32, name="bvb_sb")
            xt = persist.tile([128, n_dc * S], f16, name="xt")

            def xs(dc, sl):
                return xt[:, dc * S + sl.start: dc * S + sl.stop]
            qt = persist.tile([128, S], f16, name="qt")
            kt = persist.tile([128, S], f16, name="kt")
            # V1[kc*130 + h*65 : +65] = [V_h | ones] per key tile kc.
            v1 = persist.tile([128, n_kc * 130], f16, name="v1")
            # fp8 shadow of V for the DoubleRow pair-pops, pair-major:
            # col = j*320 + h*160 + k*80 + d for f8 pair j, half k, dim d<65.
            # The 80-col k-slots satisfy the dual-fp8 Ldweights ISA rule
            # (free-dim steps even and 16B-aligned).  Only A-kc live here.
            n_p8 = sum(1 for e in KC_PATTERN if e == 'A') * (n_kc // len(KC_PATTERN)) // 2
            # +160 pad so the k=1 shadow write can view a full 320-col window
            v1_8 = persist.tile([128, n_p8 * 320 + 160], f8, name="v1_8")

            # ACT table preload: dummy exp at t=0 hides the ~2.7us load.
            nc.vector.memset(tiny[:], 0.0)
            nc.scalar.activation(tiny_o[:], tiny[:], AF.Exp, scale=0.125)
            make_identity(nc, ident)

            # DMAs.  Issue order IS priority: each dma_start costs
            # 0.6-1.4us of Sync issue time and the DMA engines round-robin
            # fairly over everything outstanding, so: tiny weights first,
            # then x block0 as a single 3D transfer, then the later x
            # pieces (xB is also held behind xA by a deliberate 1-column
            # WAW overlap so block0+xA are never starved).
            x_src = xT[:, :].rearrange("(dc p) s -> p dc s", dc=n_dc)
            x_dst = xt[:].rearrange("p (dc s) -> p dc s", s=S)
            nc.sync.dma_start(wk_sb[:], wkT[:, :])
            nc.sync.dma_start(wq_sb[:], wqT[:, :])
            nc.sync.dma_start(x_dst[:, :, 0:512], x_src[:, :, 0:512])
            nc.sync.dma_start(x_dst[:, :, 512:1024], x_src[:, :, 512:1024])
            nc.sync.dma_start(wv_sb[:], wvT[:, :])
            nc.sync.dma_start(x_dst[:, :, 1024:2049], x_src[:, :, 1024:2049])
            nc.sync.dma_start(x_dst[:, :, 2048:S], x_src[:, :, 2048:S])
            # biases ride in the weight images as f16; widen to f32 once
            nc.vector.tensor_copy(bk_sb[:], wk_sb[:, n_dc * 128: n_dc * 128 + 1])
            nc.vector.tensor_copy(bq_sb[:], wq_sb[:, n_dc * 128: n_dc * 128 + 1])
            nc.vector.tensor_copy(bvb_sb[:], wv_sb[:, n_dc * 130: n_dc * 130 + 130])

            with (
                # 48 2KB slots: holds every deferred prologue et (44) plus
                # steady in-flight -- no prologue exp ever waits on a pop
                tc.tile_pool(name="etp", bufs=48) as etp,
                tc.tile_pool(name="outp", bufs=2) as outp,
            ):
                pools = {}  # 'st': pool for emit_norm's PE transposes

                # ---------- projection pieces ----------
                def emit_kq(dst, w_sb, b_sb, b, pool):
                    cs = slice(b * 512, (b + 1) * 512)
                    p = pool.tile([128, 512], f32, name="pp", tag="pp")
                    for dc in range(n_dc):
                        nc.tensor.matmul(
                            p[:],
                            lhsT=w_sb[:, dc * 128:(dc + 1) * 128],
                            rhs=xs(dc, cs),
                            start=(dc == 0),
                            stop=(dc == n_dc - 1),
                        )
                    nc.vector.tensor_scalar_add(dst[:, cs], p[:], b_sb[:])

                vq_done = [0]

                def is_f8_kc(kc):
                    return KC_PATTERN[kc % len(KC_PATTERN)] == 'A'

                def f8_pair(kc):
                    """(pair index j, half k) for an f8 kc under KC_PATTERN."""
                    per = len(KC_PATTERN)
                    n_a_per = sum(1 for e in KC_PATTERN if e == 'A')
                    idx = (kc // per) * n_a_per + sum(
                        1 for e in KC_PATTERN[:kc % per] if e == 'A')
                    return idx // 2, idx % 2

                def emit_vq(pool):
                    # V projection for the next 128-token tile.
                    st_ = vq_done[0]
                    ss = slice(st_ * 128, (st_ + 1) * 128)
                    p = pool.tile([128, 512], f32, name="pp", tag="pp")
                    for dc in range(n_dc):
                        nc.tensor.matmul(
                            p[:, 0:130],
                            lhsT=xs(dc, ss),
                            rhs=wv_sb[:, dc * 130:(dc + 1) * 130],
                            start=(dc == 0),
                            stop=(dc == n_dc - 1),
                        )
                    nc.vector.tensor_add(
                        v1[:, st_ * 130:(st_ + 1) * 130], p[:, 0:130], bvb_sb[:]
                    )
                    if is_f8_kc(st_):
                        # fp8 shadow for DoubleRow pops; col layout
                        # j*320 + h*160 + k*80 + d -> h-strided view
                        j, k = f8_pair(st_)
                        base = j * 320 + k * 80
                        dst = (v1_8[:, base: base + 320]
                               .rearrange("p (h c) -> p h c", h=2, c=160)
                               [:, :, 0:65])
                        src = (p[:, 0:130]
                               .rearrange("p (h c) -> p h c", h=2, c=65))
                        bsrc = (bvb_sb[:, 0:130]
                                .rearrange("p (h c) -> p h c", h=2, c=65))
                        nc.vector.tensor_add(dst, src, bsrc)
                    vq_done[0] += 1

                # ---------- attention ----------
                ev_fifo = []        # (qc, kc, h, et_tile, col_off)
                ev_left = {}        # qc -> slices not yet popped
                po_by_qc = {}

                def emit_norm(po, qc):
                    # res[:, t*128+h*64 : +64] = head h of output rows
                    # qc*512 + t*128 + [0:128); shipped as one 3D DMA
                    res = outp.tile([128, 512], f32, name="res", tag="res")
                    last = qc == n_qc - 1
                    ots = []
                    for h in range(2):
                        ot = outp.tile([128, 512], f16, name="ot", tag="ot")
                        if last and h == 0:
                            nc.scalar.copy(ot[:], po[h][:])
                        else:
                            nc.vector.tensor_copy(ot[:], po[h][:])
                        ots.append(ot)
                    for t in range(4):
                        for h in range(2):
                            if last:
                                pt = pools['st'].tile([128, 65], f16, name="pt",
                                                      tag="st")
                                nc.tensor.transpose(
                                    pt[:],
                                    ots[h][0:65, t * 128:(t + 1) * 128],
                                    ident[0:65, 0:65],
                                )
                                src = pt
                            else:
                                tp = outp.tile([128, 128], f16, name="tp",
                                               tag="tp")
                                nc.sync.dma_start_transpose(
                                    tp[:], ots[h][:, t * 128:(t + 1) * 128]
                                )
                                src = tp
                            rcp = outp.tile([128, 1], f32, name="rcp", tag="rcp")
                            nc.vector.reciprocal(rcp[:], src[:, 64:65])
                            c0 = t * 128 + h * 64
                            if last and h == 0:
                                nc.scalar.mul(
                                    res[:, c0:c0 + 64], src[:, 0:64], rcp[:],
                                )
                            else:
                                nc.vector.tensor_scalar_mul(
                                    res[:, c0:c0 + 64], src[:, 0:64], rcp[:],
                                )
                    nc.sync.dma_start(
                        out[qc * 512:(qc + 1) * 512, :]
                        .rearrange("(t p) c -> p t c", t=4),
                        res[:].rearrange("p (t c) -> p t c", t=4),
                    )

                pop_cnt = {}    # (qc, h) -> key-tiles popped so far; start/
                                # stop by count since pop order may not be
                                # kc order once f8 pairs interleave with f16.

                def pop_ev(n, ps_o, keep=0):
                    # keep: leave at least this many entries in the fifo so
                    # pops trail exp completion and never head-block the
                    # in-order PE queue waiting on an unfinished et tile.
                    popped = 0
                    while len(ev_fifo) > keep and popped < n:
                        qc, kind, kc, h, et, off = ev_fifo[0]
                        # never emit a pop ahead of its V tile: a blocked
                        # matmul would head-block the in-order PE queue
                        need_kc = kc + (2 if kind == '8' else 1)
                        if need_kc + 1 > vq_done[0] and vq_done[0] < n_kc:
                            break
                        ev_fifo.pop(0)
                        if qc not in po_by_qc:
                            po_by_qc[qc] = [
                                ps_o.tile([128, 512], f32, name=f"po{h2}",
                                          tag=f"po{h2}")
                                for h2 in range(2)
                            ]
                        po = po_by_qc[qc]
                        cnt = pop_cnt.get((qc, h), 0)
                        if kind == '8':
                            # DoubleRow fp8: one pop covers keys of kc, kc+1
                            j, _ = f8_pair(kc)
                            base = j * 320 + h * 160
                            lhsT = (v1_8[:, base: base + 160]
                                    .rearrange("p (k c) -> p k c", k=2, c=80)
                                    [:, :, 0:65])
                            rhs = (et[:]
                                   .rearrange("p (k hh c) -> p k hh c",
                                              k=2, hh=2, c=512)[:, :, h, :])
                            nkc = 2
                        else:
                            lhsT = v1[:, kc * 130 + h * 65:
                                      kc * 130 + h * 65 + 65]
                            rhs = et[:, off:off + 512]
                            nkc = 1
                        pop_cnt[(qc, h)] = cnt + nkc
                        nc.tensor.matmul(
                            po[h][0:65, :],
                            lhsT=lhsT,
                            rhs=rhs,
                            start=(cnt == 0),
                            stop=(cnt + nkc == n_kc),
                            perf_mode=(mybir.MatmulPerfMode.DoubleRow
                                       if kind == '8' else None),
                        )
                        popped += 1
                        ev_left[qc] -= nkc
                        if ev_left[qc] == 0:
                            emit_norm(po_by_qc.pop(qc), qc)

                def fill_chunk(qc, batch, st_pool):
                    """S^T matmuls for one chunk into a st_pool ring tile.
                    Slices of the same kc (h0, h1) are emitted adjacently so
                    their K=64 matmuls co-run in different PE row groups."""
                    if qc not in ev_left:
                        ev_left[qc] = 2 * n_kc
                    qs = slice(qc * 512, (qc + 1) * 512)
                    w = len(batch) * 512
                    st_ps = st_pool.tile([128, w], f32, name="st_ps", tag="st")
                    for si, (kc, h) in enumerate(batch):
                        hp = slice(h * 64, (h + 1) * 64)
                        nc.tensor.matmul(
                            st_ps[:, si * 512:(si + 1) * 512],
                            lhsT=kt[hp, kc * 128:(kc + 1) * 128],
                            rhs=qt[hp, qs],
                            start=True,
                            stop=True,
                        )
                    return (qc, batch, st_ps)

                pair_tiles = {}   # (qc, j) -> [128, 2048] f8 pair et tile

                def exp_chunk(ctx, eng, fp8=False):
                    """exp of a filled chunk on ACT ('A') or DVE ('D').
                    fp8 A-chunks write half of a [128,2048] f8 pair tile;
                    pop entries for the pair are appended on its 2nd half."""
                    qc, batch, st_ps = ctx
                    w = len(batch) * 512
                    if fp8:
                        kc = batch[0][0]
                        j, k = f8_pair(kc)
                        if (qc, j) not in pair_tiles:
                            pair_tiles[(qc, j)] = etp.tile(
                                [128, 2048], f8, name="et8", tag="et")
                        et = pair_tiles[(qc, j)]
                        nc.scalar.activation(
                            et[:, k * 1024: k * 1024 + w], st_ps[:],
                            AF.Exp, scale=0.125,
                        )
                        if k == 1:
                            pair_tiles.pop((qc, j))
                            for h in range(2):
                                ev_fifo.append((qc, '8', kc - 1, h, et, 0))
                        return
                    et = etp.tile([128, w], f16, name="et", tag="et")
                    if eng == 'A':
                        nc.scalar.activation(et[:], st_ps[:], AF.Exp,
                                             scale=0.125)
                    else:
                        nc.vector._custom_dve(
                            EXP_1OP, out=et[:], in0=st_ps[:],
                            s0=_E1_C0, s1=_E1_C1, imm2=_E1_C2,
                        )
                    for si, (kc, h) in enumerate(batch):
                        ev_fifo.append((qc, '16', kc, h, et, si * 512))

                def emit_chunk(qc, batch, eng, st_pool):
                    exp_chunk(fill_chunk(qc, batch, st_pool), eng)

                def sched(qc):
                    """32 kc-chunks (both heads, [128,1024]) in kc order;
                    engine per kc from KC_PATTERN (A-kc exp on ACT -> fp8
                    pair et; D-kc exp on DVE -> f16 et).  In the prologue
                    a few D-kc shift to ACT (f16): DVE also carries the
                    projection bias adds there."""
                    res = []
                    for kc in range(n_kc):
                        eng = KC_PATTERN[kc % len(KC_PATTERN)]
                        f8c = eng == 'A'
                        if qc < 2 and kc in PRO_ACT_D:
                            eng = 'A'
                        res.append((eng, f8c, [(kc, 0), (kc, 1)]))
                    return res

                # ---- qc0/qc1: attention + pipelined projections ----
                # No E@V pops here: the projection ring owns the two PSUM
                # banks that later hold the E@V accumulators; the 48-slot
                # et ring buffers all prologue exp outputs instead.
                with (
                    tc.tile_pool(name="pproj", bufs=2, space="PSUM") as pproj,
                    tc.tile_pool(name="ps_pro", bufs=3, space="PSUM") as ps_pro,
                ):
                    pools['st'] = ps_pro
                    # ~3.4us of dummy matmuls while x block0 is in flight:
                    # trips the PE HAM clock-gate to 2.4GHz so the first
                    # real projections don't run at half clock
                    for w in range(8):
                        wp = ps_pro.tile([128, 512], f32, name="warm", tag="st")
                        nc.tensor.matmul(
                            wp[:], lhsT=ident[:], rhs=wk_sb[:, 0:512],
                            start=True, stop=True,
                        )
                    emit_kq(kt, wk_sb, bk_sb, 0, pproj)
                    emit_kq(qt, wq_sb, bq_sb, 0, pproj)
                    # per-chunk piece schedule: K blocks ahead of their S^T
                    # use (fills of kc 4b need K_b) and behind their x DMA
                    # (timing matches the proven 22-chunk schedule, scaled
                    # 22->32); Q_b lands before its q-chunk.
                    qc0_kq = {1: "k1", 4: "k2", 7: "k3", 10: "k4", 13: "k5",
                              16: "k6", 19: "k7", 22: "q1", 25: "q2",
                              28: "q3", 31: "q4"}
                    qc1_kq = {0: "q5", 3: "q6", 6: "q7"}
                    pend = []
                    for qc, kq in ((0, qc0_kq), (1, qc1_kq)):
                        for ci, (eng, f8c, batch) in enumerate(sched(qc)):
                            pend.append(
                                (fill_chunk(qc, batch, ps_pro), eng, f8c))
                            if len(pend) >= 3:
                                ctx, e, f8e = pend.pop(0)
                                exp_chunk(ctx, e, fp8=f8e)
                            piece = kq.get(ci)
                            if piece is not None:
                                b = int(piece[1:])
                                if piece[0] == "k":
                                    emit_kq(kt, wk_sb, bk_sb, b, pproj)
                                else:
                                    emit_kq(qt, wq_sb, bq_sb, b, pproj)
                            elif not (qc == 0 and ci == 0) and vq_done[0] < n_kc:
                                emit_vq(pproj)
                    for ctx, e, f8e in pend:
                        exp_chunk(ctx, e, fp8=f8e)
                    pend = []

                # ---- qc2..qc7: decoupled ACT/DVE chunk streams ----
                # ps_o FIRST: it must overlay pproj's banks (free mid-qc1)
                # -- not ps_pro's, whose release needs every prologue exp,
                # which need et-ring slots, which need pops, which need
                # ps_o: a scheduling deadlock.
                #
                # One deep shared score ring (3 x [128,1024] = 6 banks):
                # each kc-chunk is consumed whole by ACT or DVE, fills lead
                # their exp by 2 chunks (~2us) so the in-order PE queue
                # never reaches a fill whose ring slot isn't already free,
                # and pops trail by POP_KEEP slices -- PE never waits.
                with (
                    tc.tile_pool(name="ps_o", bufs=1, space="PSUM") as ps_o,
                    tc.tile_pool(name="ps_a", bufs=3, space="PSUM") as ps_a,
                ):
                    pools['st'] = ps_a
                    chunks = [(qc, eng, f8c, batch)
                              for qc in range(2, n_qc)
                              for (eng, f8c, batch) in sched(qc)]
                    pend = []
                    for qc, eng, f8c, batch in chunks:
                        pend.append((fill_chunk(qc, batch, ps_a), eng, f8c))
                        if len(pend) >= 3:
                            ctx, e, f8e = pend.pop(0)
                            exp_chunk(ctx, e, fp8=f8e)
                        pop_ev(POP_Q, ps_o,
                               keep=(POP_KEEP_LAST if qc == n_qc - 1
                                     else POP_KEEP))
                    for ctx, e, f8e in pend:
                        exp_chunk(ctx, e, fp8=f8e)
                    pop_ev(len(ev_fifo), ps_o)
    return nc


def _shard_inputs(x, Wq, bq, Wk, bk, Wv, bv):
    """Build the 8 per-core input maps from full inputs."""
    x = np.asarray(x, dtype=np.float32)
    in_maps = []
    for c in range(N_CORES):
        b, pair = c // 4, c % 4
        rows = slice(pair * 128, (pair + 1) * 128)
        wq_s = np.asarray(Wq)[rows, :].astype(np.float32)
        wk_s = np.asarray(Wk)[rows, :].astype(np.float32)
        wv_s = np.asarray(Wv)[rows, :].astype(np.float32)
        bq_s = np.asarray(bq)[rows].astype(np.float32)
        bk_s = np.asarray(bk)[rows].astype(np.float32)
        bv_s = np.asarray(bv)[rows].astype(np.float32)

        wvT = np.zeros((D_MODEL, 130), np.float32)
        wvT[:, 0:64] = wv_s[0:64].T
        wvT[:, 65:129] = wv_s[64:128].T
        wvT = wvT.reshape(4, 128, 130).transpose(1, 0, 2).reshape(128, 520)
        wq_im = wq_s.T.reshape(4, 128, 128).transpose(1, 0, 2).reshape(128, 512)
        wk_im = wk_s.T.reshape(4, 128, 128).transpose(1, 0, 2).reshape(128, 512)
        bvb = np.zeros((128, 130), np.float32)
        bvb[:, 0:64] = bv_s[0:64]
        bvb[:, 64] = 1.0
        bvb[:, 65:129] = bv_s[64:128]
        bvb[:, 129] = 1.0
        wq_im = np.concatenate([wq_im, bq_s.reshape(128, 1)], axis=1)
        wk_im = np.concatenate([wk_im, bk_s.reshape(128, 1)], axis=1)
        wvT = np.concatenate([wvT, bvb], axis=1)

        in_maps.append({
            "xT": np.ascontiguousarray(x[c // 4].T).astype(np.float16),
            "wqT": np.ascontiguousarray(wq_im).astype(np.float16),
            "wkT": np.ascontiguousarray(wk_im).astype(np.float16),
            "wvT": wvT.astype(np.float16),
        })
    return in_maps


def _gather(results):
    B, S, D = 2, S_FULL, D_MODEL
    out = np.empty((B, S, D), np.float32)
    for c in range(N_CORES):
        b, pair = c // 4, c % 4
        out[b, :, pair * 128:(pair + 1) * 128] = results[c]["out"]
    return out


def _install_profile_hook():
    """Provide antenv.axon_hooks (missing in this image) so that
    run_bass_kernel_spmd(trace=True) can capture NTFF profiles, using the
    same ctypes path trn_boot.py would have registered."""
    import sys, types, ctypes, contextlib

    if "antenv.axon_hooks" in sys.modules:
        return
    so_path = "/opt/axon/libaxon_pjrt.so"
    mod = types.ModuleType("antenv.axon_hooks")
    state = {"hook": None}
    mod.set_axon_ntff_profile_hook = lambda h: state.__setitem__("hook", h)
    mod.get_axon_ntff_profile_hook = lambda: state["hook"]
    sys.modules["antenv.axon_hooks"] = mod
    try:
        lib = ctypes.CDLL(so_path)
        if not hasattr(lib, "axon_start_nrt_profile"):
            return
        lib.axon_start_nrt_profile.argtypes = [
            ctypes.POINTER(ctypes.c_int64), ctypes.c_size_t]
        lib.axon_start_nrt_profile.restype = ctypes.c_int64
        lib.axon_stop_nrt_profile.argtypes = [ctypes.c_char_p]
        lib.axon_stop_nrt_profile.restype = ctypes.c_int64

        @contextlib.contextmanager
        def _hook(output_dir, device_ids):
            import jax
            jax.devices()
            if device_ids:
                ids = (ctypes.c_int64 * len(device_ids))(*device_ids)
                rc = lib.axon_start_nrt_profile(ids, len(device_ids))
            else:
                rc = lib.axon_start_nrt_profile(None, 0)
            if rc != 0:
                raise RuntimeError(f"axon_start_nrt_profile rc={rc}")
            try:
                yield
            finally:
                n = lib.axon_stop_nrt_profile(str(output_dir).encode())
                print(f"profile: {n} file(s) written to {output_dir}")

        state["hook"] = _hook
    except OSError:
        pass


def kernel(x, Wq, bq, Wk, bk, Wv, bv, trace=False):
    from concourse.bass_utils import run_bass_kernel_spmd

    if trace:
        _install_profile_hook()
    if "nc" not in _cached:
        nc = build_nc(S_FULL)
        nc.finalize()
        _cached["nc"] = nc
    nc = _cached["nc"]
    in_maps = _shard_inputs(x, Wq, bq, Wk, bk, Wv, bv)
    r = run_bass_kernel_spmd(nc, in_maps, list(range(N_CORES)), trace=trace)
    _cached["last_results"] = r
    return _gather(r.results)
